# revision 5
# baseline (speedup 1.0000x reference)
"""Single-head attention (B=8, S=2048, D=128) for 8 Trainium2 NeuronCores.

Sharding: data-parallel over batch — core b computes batch element b end-to-end
(no collectives needed).

Per-core algorithm (all matmuls in fp32r: full PE rate at N=512, ~fp22 precision):
  - load x [S, D], PE-transpose to xT [D, S]
  - QT = WqT.T @ xT + bq   (= Q^T, [D, S]),  same for KT, VT  (bias per-partition)
  - V  = PE-transpose(VT)  ([S, D] natural, 128-row tiles)
  - for each q-group (512 queries):
      for each chunk of 4 k-tiles:
        scoresT[sk, sq] = KT_tile.T @ QT_group   (4 matmuls -> 4 psum banks)
        PT = exp(scale * scoresT)                (one ScalarE activation, psum->sbuf)
        oT  += V_ktile.T' @ PT                   (AV accumulate, psum [d, sq])
        den += ones.T @ PT                       (denominator rows, psum [*, sq])
      oT_norm = oT * 1/den                       (VectorE, psum->sbuf)
      out = PE-transpose(oT_norm)                ([sq, d]) -> sbuf -> DRAM
"""

import numpy as np

S = 2048
D = 128
NT = S // 128          # 16 s-tiles of 128
NG = S // 512          # 4 q-groups of 512
SCALE = float(1.0 / np.sqrt(D))

_PROGRAM = None
LAST_RESULTS = None


def _build():
    from contextlib import ExitStack

    import concourse.bass as bass
    import concourse.mybir as mybir
    import concourse.tile as tile
    from concourse import bacc

    fp32 = mybir.dt.float32
    fp32r = mybir.dt.float32r
    Exp = mybir.ActivationFunctionType.Exp

    nc = bacc.Bacc(trn_type="TRN2", target_bir_lowering=False)

    x_d = nc.dram_tensor("x", [S, D], fp32, kind="ExternalInput").ap()
    wq_d = nc.dram_tensor("wqT", [D, D], fp32, kind="ExternalInput").ap()
    wk_d = nc.dram_tensor("wkT", [D, D], fp32, kind="ExternalInput").ap()
    wv_d = nc.dram_tensor("wvT", [D, D], fp32, kind="ExternalInput").ap()
    bq_d = nc.dram_tensor("bq", [D, 1], fp32, kind="ExternalInput").ap()
    bk_d = nc.dram_tensor("bk", [D, 1], fp32, kind="ExternalInput").ap()
    bv_d = nc.dram_tensor("bv", [D, 1], fp32, kind="ExternalInput").ap()
    id_d = nc.dram_tensor("ident", [D, D], fp32, kind="ExternalInput").ap()
    out_d = nc.dram_tensor("out", [S, D], fp32, kind="ExternalOutput").ap()

    x_r = x_d.rearrange("(t p) d -> t p d", p=128)
    out_r = out_d.rearrange("(g j p) d -> g p j d", j=4, p=128)

    with tile.TileContext(nc) as tc, ExitStack() as ctx:
        singles = ctx.enter_context(tc.tile_pool(name="singles", bufs=1))
        xin = ctx.enter_context(tc.tile_pool(name="xin", bufs=3))
        ptp = ctx.enter_context(tc.tile_pool(name="pt", bufs=2))
        outp = ctx.enter_context(tc.tile_pool(name="outp", bufs=2))
        # PSUM: stage 4 banks + av 2 + tp/den 2 = 8 banks exactly
        stage_p = ctx.enter_context(tc.tile_pool(name="stage", bufs=1, space="PSUM"))
        av_p = ctx.enter_context(tc.tile_pool(name="av", bufs=2, space="PSUM"))
        tp_p = ctx.enter_context(tc.tile_pool(name="tp", bufs=2, space="PSUM"))

        # --- constants ---
        wq_sb = singles.tile([128, 128], fp32r, tag="wq")
        wk_sb = singles.tile([128, 128], fp32r, tag="wk")
        wv_sb = singles.tile([128, 128], fp32r, tag="wv")
        bq_sb = singles.tile([128, 1], fp32, tag="bq")
        bk_sb = singles.tile([128, 1], fp32, tag="bk")
        bv_sb = singles.tile([128, 1], fp32, tag="bv")
        id_sb = singles.tile([128, 128], fp32, tag="ident")
        ones_sb = singles.tile([128, 128], fp32r, tag="ones")
        for w_sb, w_d in ((wq_sb, wq_d), (wk_sb, wk_d), (wv_sb, wv_d)):
            w_stage = xin.tile([128, 128], fp32, tag="wstage")
            nc.sync.dma_start(out=w_stage, in_=w_d)
            nc.vector.tensor_copy(w_sb, w_stage)
        nc.sync.dma_start(out=bq_sb, in_=bq_d)
        nc.sync.dma_start(out=bk_sb, in_=bk_d)
        nc.sync.dma_start(out=bv_sb, in_=bv_d)
        nc.sync.dma_start(out=id_sb, in_=id_d)
        ones_stage = xin.tile([128, 128], fp32, tag="wstage")
        nc.vector.memset(ones_stage, 1.0)
        nc.vector.tensor_copy(ones_sb, ones_stage)

        # --- persistent big sbuf tensors ---
        xT_sb = singles.tile([128, S], fp32r, tag="xT")   # [d, s]
        qT_sb = singles.tile([128, S], fp32r, tag="qT")   # [e, s]
        kT_sb = singles.tile([128, S], fp32r, tag="kT")   # [e, s]
        vT_sb = singles.tile([128, S], fp32, tag="vT")   # [e, s]
        v_sb = singles.tile([128, S], fp32r, tag="v")     # 16 tiles of [s(128), d]

        # --- load x and transpose to xT ---
        for c in range(4):
            tpt = tp_p.tile([128, 512], fp32, tag="tp")
            for j in range(4):
                t = 4 * c + j
                x_t = xin.tile([128, 128], fp32, tag="x")
                nc.sync.dma_start(out=x_t, in_=x_r[t])
                nc.tensor.matmul(
                    tpt[:, 128 * j:128 * (j + 1)], lhsT=x_t, rhs=id_sb,
                    is_transpose=True, start=(j == 0), stop=(j == 3),
                )
            nc.vector.tensor_copy(xT_sb[:, 512 * c:512 * (c + 1)], tpt)

        # --- projections QT/KT/VT = W.T.T @ xT + b ---
        for c in range(4):
            sl = slice(512 * c, 512 * (c + 1))
            for w_sb, b_sb, dst in (
                (wq_sb, bq_sb, qT_sb), (wk_sb, bk_sb, kT_sb), (wv_sb, bv_sb, vT_sb),
            ):
                pp = av_p.tile([128, 512], fp32, tag="av")
                nc.tensor.matmul(pp, lhsT=w_sb, rhs=xT_sb[:, sl],
                                 start=True, stop=True)
                nc.vector.tensor_scalar_add(dst[:, sl], pp, b_sb)

        # --- V natural layout: transpose VT tiles ---
        for c in range(4):
            tpt = tp_p.tile([128, 512], fp32, tag="tp")
            for j in range(4):
                t = 4 * c + j
                nc.tensor.matmul(
                    tpt[:, 128 * j:128 * (j + 1)],
                    lhsT=vT_sb[:, 128 * t:128 * (t + 1)], rhs=id_sb,
                    is_transpose=True, start=(j == 0), stop=(j == 3),
                )
            nc.vector.tensor_copy(v_sb[:, 512 * c:512 * (c + 1)], tpt)

        # --- main attention loop over q-groups ---
        for g in range(NG):
            gsl = slice(512 * g, 512 * (g + 1))
            av = av_p.tile([128, 512], fp32, tag="av")    # oT accumulator [d, sq]
            den = tp_p.tile([128, 512], fp32, tag="tp")   # denominator rows
            for c in range(4):
                st = stage_p.tile([128, 2048], fp32, tag="stage")
                for j in range(4):
                    kt = 4 * c + j
                    nc.tensor.matmul(
                        st[:, 512 * j:512 * (j + 1)],
                        lhsT=kT_sb[:, 128 * kt:128 * (kt + 1)],
                        rhs=qT_sb[:, gsl],
                        start=True, stop=True,
                    )
                pt = ptp.tile([128, 2048], fp32r, tag="pt")
                nc.scalar.activation(pt, st, Exp, scale=SCALE)
                for j in range(4):
                    kt = 4 * c + j
                    psl = slice(512 * j, 512 * (j + 1))
                    nc.tensor.matmul(
                        av, lhsT=v_sb[:, 128 * kt:128 * (kt + 1)],
                        rhs=pt[:, psl],
                        start=(kt == 0), stop=(kt == 15),
                    )
                    nc.tensor.matmul(
                        den, lhsT=ones_sb, rhs=pt[:, psl],
                        start=(kt == 0), stop=(kt == 15),
                    )
            recip = outp.tile([128, 512], fp32, tag="recip")
            nc.vector.reciprocal(recip, den)
            onorm = outp.tile([128, 512], fp32, tag="onorm")
            nc.vector.tensor_mul(onorm, av, recip)
            tpo = tp_p.tile([128, 512], fp32, tag="tp")
            for j in range(4):
                nc.tensor.matmul(
                    tpo[:, 128 * j:128 * (j + 1)],
                    lhsT=onorm[:, 128 * j:128 * (j + 1)], rhs=id_sb,
                    is_transpose=True, start=(j == 0), stop=(j == 3),
                )
            osb = outp.tile([128, 512], fp32, tag="osb")
            nc.vector.tensor_copy(osb, tpo)
            nc.sync.dma_start(
                out=out_r[g], in_=osb.rearrange("p (j d) -> p j d", j=4)
            )

    nc.compile()
    return nc


def _get_program():
    global _PROGRAM
    if _PROGRAM is None:
        _PROGRAM = _build()
    return _PROGRAM


def _ensure_axon_hooks():
    """bass_utils imports antenv.axon_hooks when tracing; provide a stub if
    the image's antenv lacks it (hook defaults to None => tracing skipped)."""
    import sys
    import types
    try:
        import antenv.axon_hooks  # noqa: F401
        return
    except ImportError:
        pass
    import antenv
    m = types.ModuleType("antenv.axon_hooks")
    m._hook = None
    def _set(h):
        m._hook = h
    def _get():
        return m._hook
    m.set_axon_ntff_profile_hook = _set
    m.get_axon_ntff_profile_hook = _get
    sys.modules["antenv.axon_hooks"] = m
    antenv.axon_hooks = m


def kernel(input1, Wq, bq, Wk, bk, Wv, bv):
    global LAST_RESULTS
    _ensure_axon_hooks()
    from concourse.bass_utils import run_bass_kernel_spmd

    nc = _get_program()

    input1 = np.ascontiguousarray(np.asarray(input1, dtype=np.float32))
    common = {
        "wqT": np.ascontiguousarray(np.asarray(Wq, np.float32).T),
        "wkT": np.ascontiguousarray(np.asarray(Wk, np.float32).T),
        "wvT": np.ascontiguousarray(np.asarray(Wv, np.float32).T),
        "bq": np.asarray(bq, np.float32).reshape(D, 1).copy(),
        "bk": np.asarray(bk, np.float32).reshape(D, 1).copy(),
        "bv": np.asarray(bv, np.float32).reshape(D, 1).copy(),
        "ident": np.eye(D, dtype=np.float32),
    }
    in_maps = [dict(common, x=input1[b]) for b in range(8)]
    res = run_bass_kernel_spmd(nc, in_maps, core_ids=list(range(8)))
    LAST_RESULTS = res
    return np.stack([r["out"] for r in res.results], axis=0)


# revision 12
# speedup vs baseline: 1.1806x; 1.1806x over previous
"""Single-head attention (B=8, S=2048, D=128) for 8 Trainium2 NeuronCores.

Sharding: data-parallel over batch — core b computes batch element b end-to-end
(no collectives needed).

Per-core algorithm (all matmuls in fp32r: full PE rate at N=512, ~fp22 precision):
  - load x [S, D], PE-transpose to xT [D, S]
  - QT = WqT.T @ xT + bq   (= Q^T, [D, S]),  same for KT, VT  (bias per-partition)
  - V  = PE-transpose(VT)  ([S, D] natural, 128-row tiles)
  - for each q-group (512 queries):
      for each chunk of 4 k-tiles:
        scoresT[sk, sq] = KT_tile.T @ QT_group   (4 matmuls -> 4 psum banks)
        PT = exp(scale * scoresT)                (one ScalarE activation, psum->sbuf)
        oT  += V_ktile.T' @ PT                   (AV accumulate, psum [d, sq])
        den += ones.T @ PT                       (denominator rows, psum [*, sq])
      oT_norm = oT * 1/den                       (VectorE, psum->sbuf)
      out = PE-transpose(oT_norm)                ([sq, d]) -> sbuf -> DRAM
"""

import numpy as np

S = 2048
D = 128
NT = S // 128          # 16 s-tiles of 128
NG = S // 512          # 4 q-groups of 512
SCALE = float(1.0 / np.sqrt(D))

_PROGRAM = None
LAST_RESULTS = None


def _build():
    from contextlib import ExitStack

    import concourse.bass as bass
    import concourse.mybir as mybir
    import concourse.tile as tile
    from concourse import bacc

    fp32 = mybir.dt.float32
    fp32r = mybir.dt.float32r
    bf16 = mybir.dt.bfloat16
    Exp = mybir.ActivationFunctionType.Exp

    nc = bacc.Bacc(trn_type="TRN2", target_bir_lowering=False)

    x_d = nc.dram_tensor("x", [S, D], fp32, kind="ExternalInput").ap()
    wq_d = nc.dram_tensor("wqT", [D, D], fp32, kind="ExternalInput").ap()
    wk_d = nc.dram_tensor("wkT", [D, D], fp32, kind="ExternalInput").ap()
    wv_d = nc.dram_tensor("wvT", [D, D], fp32, kind="ExternalInput").ap()
    bq_d = nc.dram_tensor("bq", [D, 1], fp32, kind="ExternalInput").ap()
    bk_d = nc.dram_tensor("bk", [D, 1], fp32, kind="ExternalInput").ap()
    bv_d = nc.dram_tensor("bv", [D, 1], fp32, kind="ExternalInput").ap()
    id_d = nc.dram_tensor("ident", [D, D], fp32, kind="ExternalInput").ap()
    sel_d = nc.dram_tensor("sel", [D, 4], fp32, kind="ExternalInput").ap()
    out_d = nc.dram_tensor("out", [S, D], fp32, kind="ExternalOutput").ap()

    x_r = x_d.rearrange("(t p) d -> t p d", p=128)
    out_r = out_d.rearrange("(g j p) d -> g p j d", j=4, p=128)

    with tile.TileContext(nc) as tc, ExitStack() as ctx:
        singles = ctx.enter_context(tc.tile_pool(name="singles", bufs=1))
        xin = ctx.enter_context(tc.tile_pool(name="xin", bufs=3))
        ptp = ctx.enter_context(tc.tile_pool(name="pt", bufs=2))
        outp = ctx.enter_context(tc.tile_pool(name="outp", bufs=2))
        # PSUM: stage 4 banks + av 2 + tp/den 2 = 8 banks exactly
        stage_p = ctx.enter_context(tc.tile_pool(name="stage", bufs=1, space="PSUM"))
        av_p = ctx.enter_context(tc.tile_pool(name="av", bufs=2, space="PSUM"))
        tp_p = ctx.enter_context(tc.tile_pool(name="tp", bufs=2, space="PSUM"))

        # --- constants ---
        wq_sb = singles.tile([128, 128], fp32r, tag="wq")
        wk_sb = singles.tile([128, 128], fp32r, tag="wk")
        wv_sb = singles.tile([128, 128], fp32r, tag="wv")
        bq_sb = singles.tile([128, 1], fp32, tag="bq")
        bk_sb = singles.tile([128, 1], fp32, tag="bk")
        bv_sb = singles.tile([128, 1], fp32, tag="bv")
        id_sb = singles.tile([128, 128], fp32, tag="ident")
        sel_sb = singles.tile([128, 4], fp32r, tag="sel")
        ones_sb = singles.tile([128, 128], bf16, tag="ones")
        for w_sb, w_d in ((wq_sb, wq_d), (wk_sb, wk_d), (wv_sb, wv_d)):
            w_stage = xin.tile([128, 128], fp32, tag="wstage")
            nc.sync.dma_start(out=w_stage, in_=w_d)
            nc.vector.tensor_copy(w_sb, w_stage)
        nc.sync.dma_start(out=bq_sb, in_=bq_d)
        nc.sync.dma_start(out=bk_sb, in_=bk_d)
        nc.sync.dma_start(out=bv_sb, in_=bv_d)
        nc.sync.dma_start(out=id_sb, in_=id_d)
        sel_stage = xin.tile([128, 4], fp32, tag="selstage")
        nc.sync.dma_start(out=sel_stage, in_=sel_d)
        nc.vector.tensor_copy(sel_sb, sel_stage)
        ones_stage = xin.tile([128, 128], fp32, tag="wstage")
        nc.vector.memset(ones_stage, 1.0)
        nc.vector.tensor_copy(ones_sb, ones_stage)

        # --- persistent big sbuf tensors ---
        xT_sb = singles.tile([128, S], fp32r, tag="xT")   # [d, s]
        qT_sb = singles.tile([128, S], fp32r, tag="qT")   # [e, s]
        kT_sb = singles.tile([128, S], fp32r, tag="kT")   # [e, s]
        vT_sb = singles.tile([128, S], fp32, tag="vT")   # [e, s]
        v_sb = singles.tile([128, S], bf16, tag="v")     # 16 tiles of [s(128), d]

        # --- load x and transpose to xT ---
        for c in range(4):
            tpt = tp_p.tile([128, 512], fp32, tag="tp")
            for j in range(4):
                t = 4 * c + j
                x_t = xin.tile([128, 128], fp32, tag="x")
                nc.sync.dma_start(out=x_t, in_=x_r[t])
                nc.tensor.matmul(
                    tpt[:, 128 * j:128 * (j + 1)], lhsT=x_t, rhs=id_sb,
                    is_transpose=True, start=(j == 0), stop=(j == 3),
                )
            nc.vector.tensor_copy(xT_sb[:, 512 * c:512 * (c + 1)], tpt)

        # --- projections QT/KT/VT = W.T.T @ xT + b ---
        for c in range(4):
            sl = slice(512 * c, 512 * (c + 1))
            for w_sb, b_sb, dst in (
                (wq_sb, bq_sb, qT_sb), (wk_sb, bk_sb, kT_sb), (wv_sb, bv_sb, vT_sb),
            ):
                pp = av_p.tile([128, 512], fp32, tag="av")
                nc.tensor.matmul(pp, lhsT=w_sb, rhs=xT_sb[:, sl],
                                 start=True, stop=True)
                nc.vector.tensor_scalar_add(dst[:, sl], pp, b_sb)

        # --- V natural layout: transpose VT tiles ---
        for c in range(4):
            tpt = tp_p.tile([128, 512], fp32, tag="tp")
            for j in range(4):
                t = 4 * c + j
                nc.tensor.matmul(
                    tpt[:, 128 * j:128 * (j + 1)],
                    lhsT=vT_sb[:, 128 * t:128 * (t + 1)], rhs=id_sb,
                    is_transpose=True, start=(j == 0), stop=(j == 3),
                )
            nc.vector.tensor_copy(v_sb[:, 512 * c:512 * (c + 1)], tpt)

        # --- main attention loop over q-groups ---
        for g in range(NG):
            gsl = slice(512 * g, 512 * (g + 1))
            av = av_p.tile([128, 512], fp32, tag="av")    # oT accumulator [d, sq]
            den = tp_p.tile([128, 512], fp32, tag="tp")   # 4 col-packed strips
            for c in range(4):
                st = stage_p.tile([128, 2048], fp32, tag="stage")
                for j in range(4):
                    kt = 4 * c + j
                    nc.tensor.matmul(
                        st[:, 512 * j:512 * (j + 1)],
                        lhsT=kT_sb[:, 128 * kt:128 * (kt + 1)],
                        rhs=qT_sb[:, gsl],
                        start=True, stop=True,
                    )
                pt = ptp.tile([128, 2048], bf16, tag="pt")
                nc.scalar.activation(pt, st, Exp, scale=SCALE)
                for j in range(4):
                    kt = 4 * c + j
                    nc.tensor.matmul(
                        av, lhsT=v_sb[:, 128 * kt:128 * (kt + 1)],
                        rhs=pt[:, 512 * j:512 * (j + 1)],
                        start=(kt == 0), stop=(kt == 15),
                    )
                # denominator strip j accumulates k-tiles {j, j+4, j+8, j+12};
                # the 4 M=1 matmuls go to distinct column groups back-to-back
                # so they run concurrently in the PE array.
                for j in range(4):
                    nc.tensor.matmul(
                        den[32 * j:32 * (j + 1), :],
                        lhsT=ones_sb[:, 0:32], rhs=pt[:, 512 * j:512 * (j + 1)],
                        start=(c == 0), stop=(c == 3),
                        tile_position=(0, 32 * j),
                        skip_group_check=True,
                    )
            # strips -> sbuf (one copy), then selector matmuls sum the 4 strip
            # partitions AND land q on partitions: denT[q, j] = sum_p sel[p]*den[p, 128j+q]
            den_fs = outp.tile([128, 512], fp32r, tag="denfs")
            nc.vector.tensor_copy(den_fs, den)
            denT = tp_p.tile([128, 16], fp32, tag="tp")
            for j in range(4):
                nc.tensor.matmul(
                    denT[:, 4 * j:4 * (j + 1)], lhsT=den_fs[:, 128 * j:128 * (j + 1)],
                    rhs=sel_sb, start=(j == 0), stop=(j == 3),
                )
            recip = outp.tile([128, 16], fp32, tag="recip")
            nc.vector.reciprocal(recip, denT)
            # unnormalized oT -> sbuf, transpose back to [sq, d], scale by 1/den
            oT_sb = outp.tile([128, 512], fp32, tag="oTsb")
            nc.vector.tensor_copy(oT_sb, av)
            tpo = tp_p.tile([128, 512], fp32, tag="tp")
            for j in range(4):
                nc.tensor.matmul(
                    tpo[:, 128 * j:128 * (j + 1)],
                    lhsT=oT_sb[:, 128 * j:128 * (j + 1)], rhs=id_sb,
                    is_transpose=True, start=(j == 0), stop=(j == 3),
                )
            osb = outp.tile([128, 512], fp32, tag="osb")
            for j in range(4):
                nc.vector.tensor_scalar_mul(
                    osb[:, 128 * j:128 * (j + 1)],
                    tpo[:, 128 * j:128 * (j + 1)], recip[:, 4 * j:4 * j + 1],
                )
            nc.sync.dma_start(
                out=out_r[g], in_=osb.rearrange("p (j d) -> p j d", j=4)
            )

    nc.compile()
    return nc


def _get_program():
    global _PROGRAM
    if _PROGRAM is None:
        _PROGRAM = _build()
    return _PROGRAM


def _ensure_axon_hooks():
    """bass_utils imports antenv.axon_hooks when tracing; provide a stub if
    the image's antenv lacks it (hook defaults to None => tracing skipped)."""
    import sys
    import types
    try:
        import antenv.axon_hooks  # noqa: F401
        return
    except ImportError:
        pass
    import antenv
    m = types.ModuleType("antenv.axon_hooks")
    m._hook = None
    def _set(h):
        m._hook = h
    def _get():
        return m._hook
    m.set_axon_ntff_profile_hook = _set
    m.get_axon_ntff_profile_hook = _get
    sys.modules["antenv.axon_hooks"] = m
    antenv.axon_hooks = m


def kernel(input1, Wq, bq, Wk, bk, Wv, bv):
    global LAST_RESULTS
    _ensure_axon_hooks()
    from concourse.bass_utils import run_bass_kernel_spmd

    nc = _get_program()

    input1 = np.ascontiguousarray(np.asarray(input1, dtype=np.float32))
    common = {
        "wqT": np.ascontiguousarray(np.asarray(Wq, np.float32).T),
        "wkT": np.ascontiguousarray(np.asarray(Wk, np.float32).T),
        "wvT": np.ascontiguousarray(np.asarray(Wv, np.float32).T),
        "bq": np.asarray(bq, np.float32).reshape(D, 1).copy(),
        "bk": np.asarray(bk, np.float32).reshape(D, 1).copy(),
        "bv": np.asarray(bv, np.float32).reshape(D, 1).copy(),
        "ident": np.eye(D, dtype=np.float32),
        "sel": np.tile(np.array([1.0 if p % 32 == 0 else 0.0 for p in range(D)],
                        np.float32).reshape(D, 1), (1, 4)),
    }
    in_maps = [dict(common, x=input1[b]) for b in range(8)]
    res = run_bass_kernel_spmd(nc, in_maps, core_ids=list(range(8)))
    LAST_RESULTS = res
    return np.stack([r["out"] for r in res.results], axis=0)


# revision 34
# speedup vs baseline: 1.8332x; 1.5528x over previous
"""Single-head attention (B=8, S=2048, D=128) on 8 Trainium2 NeuronCores.

Sharding: data-parallel over batch — core b computes batch element b end to end
(no collectives). kernel() takes full inputs, returns the full output.

Per-core algorithm (Tile framework, one NEFF run SPMD on 8 cores):
  - x is DMA'd with 16 consecutive rows per partition (8 KB contiguous per
    partition, near-peak DMA). This perfectly-shuffles the sequence axis
    (s = 16p + t); attention is permutation-equivariant, so the output DMA
    simply inverts the shuffle.
  - xT = PE-transpose(x tiles); QT/KT = W.T @ xT + b as [d,s] (bf16), V
    directly as [s,d] tiles from xT (bf16, bias via broadcast add).
  - Main loop, software-pipelined over 8 chunks (2 k-tiles) per q-group:
      scoresT[sk,sq] = KT_kt.T @ QT_g   (bf16 matmuls, N=512, psum fp32,
                                         double-buffered 2-bank stage slots)
      PT = exp(scale*scoresT)           (one ScalarE activation per chunk,
                                         psum->sbuf, bf16 out)
      oT += V_kt.T' @ PT                (AV accumulate [d,sq] in psum)
      den: ones.T @ PT                  (4 M=32 col-group-packed matmuls per
                                         2 chunks, concurrent in PE array)
    PE issues scores one chunk ahead of AV/den so it never head-of-line
    blocks on the exp.
  - Epilogue per group: den strips -> sbuf, selector matmuls (sum strips AND
    put q on partitions), one reciprocal [128,16], PE-transpose of oT back to
    [sq,d], per-partition scale by 1/den during the psum->sbuf copy, DMA out.

Numerics: scores/AV in bf16 with fp32 psum accumulation (rel err ~2.6e-3 vs
fp32 reference; exp/softmax denominators in fp32, den reduction in fp32r).
Set USE_BF16_QK=False for fp32r (~fp22) scores (~9e-4 rel err, ~15 us slower).
"""

import numpy as np

S = 2048
D = 128
USE_BF16_QK = True
NT = S // 128          # 16 s-tiles of 128
NG = S // 512          # 4 q-groups of 512
SCALE = float(1.0 / np.sqrt(D))

_PROGRAM = None
LAST_RESULTS = None


def _build():
    from contextlib import ExitStack

    import concourse.bass as bass
    import concourse.mybir as mybir
    import concourse.tile as tile
    from concourse import bacc

    fp32 = mybir.dt.float32
    fp32r = mybir.dt.float32r
    bf16 = mybir.dt.bfloat16
    qkdt = bf16 if USE_BF16_QK else fp32r
    Exp = mybir.ActivationFunctionType.Exp

    nc = bacc.Bacc(trn_type="TRN2", target_bir_lowering=False)

    x_d = nc.dram_tensor("x", [S, D], fp32, kind="ExternalInput").ap()
    w_d = nc.dram_tensor("w3", [D, 3 * D], fp32, kind="ExternalInput").ap()
    # consts layout: [bq | bk | bv | sel(4) | ident(128) | bv_bcast(4x128)] = [128, 647]
    c_d = nc.dram_tensor("consts", [D, 647], fp32, kind="ExternalInput").ap()
    out_d = nc.dram_tensor("out", [S, D], fp32, kind="ExternalOutput").ap()

    # x loaded with 16 consecutive rows per partition (8 KB contiguous per
    # partition -> near-peak DMA). This applies the perfect-shuffle permutation
    # s = 16*p + t to the sequence axis; attention is permutation-equivariant,
    # so we simply invert it when storing the output.
    x_r = x_d.rearrange("(p r) d -> p r d", p=128)
    out_r = out_d.rearrange("(p r) d -> p r d", p=128)

    with tile.TileContext(nc) as tc, ExitStack() as ctx:
        singles = ctx.enter_context(tc.tile_pool(name="singles", bufs=1))
        xin = ctx.enter_context(tc.tile_pool(name="xin", bufs=3))
        ptp = ctx.enter_context(tc.tile_pool(name="pt", bufs=3))
        outp = ctx.enter_context(tc.tile_pool(name="outp", bufs=2))
        # PSUM: stage 4 banks + av 2 + tp/den 2 = 8 banks exactly
        stage_p = ctx.enter_context(tc.tile_pool(name="stage", bufs=1, space="PSUM"))
        av_p = ctx.enter_context(tc.tile_pool(name="av", bufs=1, space="PSUM"))
        tp_p = ctx.enter_context(tc.tile_pool(name="tp", bufs=1, space="PSUM"))

        # --- constants (small consts DMA first: identity gates the transposes) ---
        consts_sb = singles.tile([128, 647], fp32, tag="consts")
        nc.sync.dma_start(out=consts_sb[:, 0:135], in_=c_d[:, 0:135])
        nc.gpsimd.dma_start(out=consts_sb[:, 135:647], in_=c_d[:, 135:647])
        bq_sb = consts_sb[:, 0:1]
        bk_sb = consts_sb[:, 1:2]
        bv_sb = consts_sb[:, 2:3]
        id_sb = consts_sb[:, 7:135]
        bvb_sb = consts_sb[:, 135:647]
        sel_sb = singles.tile([128, 4], fp32r, tag="sel")
        nc.vector.tensor_copy(sel_sb, consts_sb[:, 3:7])
        ones_sb = singles.tile([128, 128], bf16, tag="ones")
        id16_sb = singles.tile([128, 128], bf16, tag="id16")
        ones_stage = xin.tile([128, 128], fp32, tag="wstage")
        nc.vector.memset(ones_stage, 1.0)
        nc.vector.tensor_copy(ones_sb, ones_stage)
        nc.vector.tensor_copy(id16_sb, id_sb)

        # x: 2 half-DMAs (contiguous 4 KB per partition each)
        x_q = []
        for h in range(2):
            xh = singles.tile([128, 8, 128], fp32, tag=f"xh{h}", name=f"xh_{h}")
            nc.sync.dma_start(out=xh, in_=x_r[:, 8 * h:8 * (h + 1), :])
            x_q.append(xh)

        w3_stage = singles.tile([128, 384], fp32, tag="w3stage")
        nc.gpsimd.dma_start(out=w3_stage, in_=w_d)
        w3_sb = singles.tile([128, 384], qkdt, tag="w3")
        nc.vector.tensor_copy(w3_sb, w3_stage)
        wq_sb = w3_sb[:, 0:128]
        wk_sb = w3_sb[:, 128:256]
        wv_sb = w3_sb[:, 256:384]

        # --- persistent big sbuf tensors ---
        xT_sb = singles.tile([128, S], qkdt, tag="xT")   # [d, s]
        qT_sb = singles.tile([128, S], qkdt, tag="qT")   # [e, s]
        kT_sb = singles.tile([128, S], qkdt, tag="kT")   # [e, s]
        v_sb = singles.tile([128, S], bf16, tag="v")     # 16 tiles of [s(128), d]

        # per chunk-of-4-tiles: cast x to bf16, transpose, project QT/KT, compute V
        for c in range(4):
            tpt = tp_p.tile([128, 512], fp32, tag=f"tp{c % 2}", name=f"tptx_{c}")
            for j in range(4):
                t = 4 * c + j
                nc.tensor.matmul(
                    tpt[:, 128 * j:128 * (j + 1)], lhsT=x_q[t // 8][:, t % 8, :],
                    rhs=id_sb,
                    is_transpose=True, start=(j == 0), stop=(j == 3),
                )
            nc.vector.tensor_copy(xT_sb[:, 512 * c:512 * (c + 1)], tpt)
        for c in range(4):
            sl = slice(512 * c, 512 * (c + 1))
            for wi, (w_sb, b_sb, dst) in enumerate((
                (wq_sb, bq_sb, qT_sb), (wk_sb, bk_sb, kT_sb),
            )):
                pp = av_p.tile([128, 512], fp32, tag=f"av{wi % 2}", name=f"pp_{c}_{wi}")
                nc.tensor.matmul(pp, lhsT=w_sb, rhs=xT_sb[:, sl],
                                 start=True, stop=True)
                nc.vector.tensor_scalar_add(dst[:, sl], pp, b_sb)
        for c in range(4):
            tpv = tp_p.tile([128, 512], fp32, tag=f"tp{(c + 1) % 2}", name=f"tptv_{c}")
            for j in range(4):
                t = 4 * c + j
                nc.tensor.matmul(
                    tpv[:, 128 * j:128 * (j + 1)],
                    lhsT=xT_sb[:, 128 * t:128 * (t + 1)], rhs=wv_sb,
                    start=(j == 0), stop=(j == 3), skip_group_check=True,
                )
            nc.vector.tensor_add(v_sb[:, 512 * c:512 * (c + 1)], tpv, bvb_sb)

        # --- main attention loop, software-pipelined over 2-k-tile chunks ---
        # Per step: PE issues scores(chunk i) first (stage is double-buffered,
        # so always ready), then AV+den for chunk i-1 (whose exp finished during
        # the previous step). ScalarE exp of chunk i overlaps AV/den of i-1.
        NCH = 8                       # chunks per group, 2 k-tiles each
        chunks = [(g, c) for g in range(NG) for c in range(NCH)]
        av = den = None
        avs, dens, pts = {}, {}, {}

        def issue_scores(g, c):
            st = stage_p.tile([128, 1024], fp32, tag=f"stage{(g * NCH + c) % 2}",
                              name=f"st_{g}_{c}")
            with nc.named_scope("scores"):
                for j in range(2):
                    kt = 2 * c + j
                    nc.tensor.matmul(
                        st[:, 512 * j:512 * (j + 1)],
                        lhsT=kT_sb[:, 128 * kt:128 * (kt + 1)],
                        rhs=qT_sb[:, 512 * g:512 * (g + 1)],
                        start=True, stop=True,
                    )
            pt = ptp.tile([128, 1024], bf16, tag="pt")
            with nc.named_scope("exp"):
                nc.scalar.activation(pt, st, Exp, scale=SCALE)
            return pt

        def issue_den_quad(g, c0):
            # quad covers chunks c0, c0+1 (k-tiles 2*c0 .. 2*c0+3), both pt
            # tiles already materialized -> 4 back-to-back col-group matmuls
            # run concurrently in the PE array.
            with nc.named_scope("den"):
                for q in range(4):
                    kt = 2 * c0 + q
                    ptq, jq = pts[g, c0 + q // 2], kt % 2
                    strip = kt % 4
                    nc.tensor.matmul(
                        dens[g][32 * strip:32 * (strip + 1), :],
                        lhsT=ones_sb[:, 0:32],
                        rhs=ptq[:, 512 * jq:512 * (jq + 1)],
                        start=(c0 == 0), stop=(c0 == NCH - 2),
                        tile_position=(0, 32 * strip),
                        skip_group_check=True,
                    )

        def issue_avden(g, c, pt):
            pts[g, c] = pt
            with nc.named_scope("av"):
                for j in range(2):
                    kt = 2 * c + j
                    nc.tensor.matmul(
                        avs[g], lhsT=v_sb[:, 128 * kt:128 * (kt + 1)],
                        rhs=pt[:, 512 * j:512 * (j + 1)],
                        start=(kt == 0), stop=(kt == 15),
                    )
            if c % 2 == 0 and c > 0:
                issue_den_quad(g, c - 2)

        def epilogue(g):
            av, den = avs.pop(g), dens.pop(g)
            with nc.named_scope("epi"):
                den_fs = outp.tile([128, 512], fp32r, tag="denfs")
                nc.vector.tensor_copy(den_fs, den)
                denT = tp_p.tile([128, 16], fp32, tag=f"tp{(g + 1) % 2}", name=f"denT_{g}")
                for j in range(4):
                    nc.tensor.matmul(
                        denT[:, 4 * j:4 * (j + 1)],
                        lhsT=den_fs[:, 128 * j:128 * (j + 1)],
                        rhs=sel_sb, start=(j == 0), stop=(j == 3),
                    )
                recip = outp.tile([128, 16], fp32, tag="recip")
                nc.vector.reciprocal(recip, denT)
                oT_sb = outp.tile([128, 512], bf16, tag="oTsb")
                nc.vector.tensor_copy(oT_sb, av)
                tpo = tp_p.tile([128, 512], bf16, tag=f"tp{g % 2}", name=f"tpo_{g}")
                for j in range(4):
                    nc.tensor.matmul(
                        tpo[:, 128 * j:128 * (j + 1)],
                        lhsT=oT_sb[:, 128 * j:128 * (j + 1)], rhs=id16_sb,
                        is_transpose=True, start=(j == 0), stop=(j == 3),
                    )
                osb = outp.tile([128, 512], fp32, tag="osb")
                for j in range(4):
                    nc.vector.tensor_scalar_mul(
                        osb[:, 128 * j:128 * (j + 1)],
                        tpo[:, 128 * j:128 * (j + 1)], recip[:, 4 * j:4 * j + 1],
                    )
                nc.sync.dma_start(
                    out=out_r[:, 4 * g:4 * (g + 1), :],
                    in_=osb.rearrange("p (j d) -> p j d", j=4),
                )

        prev = None
        for g, c in chunks:
            if c == 0:
                avs[g] = av_p.tile([128, 512], fp32, tag=f"av{g % 2}", name=f"av_{g}")
                dens[g] = tp_p.tile([128, 512], fp32, tag=f"tp{g % 2}", name=f"den_{g}")
            pt = issue_scores(g, c)
            if prev is not None:
                issue_avden(*prev)
                if prev[1] == NCH - 1:
                    issue_den_quad(prev[0], NCH - 2)
                    epilogue(prev[0])
            prev = (g, c, pt)
        issue_avden(*prev)
        issue_den_quad(prev[0], NCH - 2)
        epilogue(prev[0])

    nc.compile()
    return nc


def _get_program():
    global _PROGRAM
    if _PROGRAM is None:
        _PROGRAM = _build()
    return _PROGRAM


def _ensure_axon_hooks():
    """bass_utils imports antenv.axon_hooks when tracing; provide a stub if
    the image's antenv lacks it (hook defaults to None => tracing skipped)."""
    import sys
    import types
    try:
        import antenv.axon_hooks  # noqa: F401
        return
    except ImportError:
        pass
    import antenv
    m = types.ModuleType("antenv.axon_hooks")
    m._hook = None
    def _set(h):
        m._hook = h
    def _get():
        return m._hook
    m.set_axon_ntff_profile_hook = _set
    m.get_axon_ntff_profile_hook = _get
    sys.modules["antenv.axon_hooks"] = m
    antenv.axon_hooks = m


def kernel(input1, Wq, bq, Wk, bk, Wv, bv):
    global LAST_RESULTS
    _ensure_axon_hooks()
    from concourse.bass_utils import run_bass_kernel_spmd

    nc = _get_program()

    input1 = np.ascontiguousarray(np.asarray(input1, dtype=np.float32))
    w3 = np.concatenate([np.asarray(W, np.float32).T for W in (Wq, Wk, Wv)],
                        axis=1)
    sel = np.tile(np.array([1.0 if p % 32 == 0 else 0.0 for p in range(D)],
                  np.float32).reshape(D, 1), (1, 4))
    consts = np.concatenate([
        np.asarray(bq, np.float32).reshape(D, 1),
        np.asarray(bk, np.float32).reshape(D, 1),
        np.asarray(bv, np.float32).reshape(D, 1),
        sel,
        np.eye(D, dtype=np.float32),
        np.tile(np.asarray(bv, np.float32).reshape(1, D), (D, 4)),
    ], axis=1)
    common = {
        "w3": np.ascontiguousarray(w3),
        "consts": np.ascontiguousarray(consts),
    }
    in_maps = [dict(common, x=input1[b]) for b in range(8)]
    res = run_bass_kernel_spmd(nc, in_maps, core_ids=list(range(8)))
    LAST_RESULTS = res
    return np.stack([r["out"] for r in res.results], axis=0)


# revision 35
# speedup vs baseline: 1.8354x; 1.0012x over previous
"""Single-head attention (B=8, S=2048, D=128) on 8 Trainium2 NeuronCores.

Sharding: data-parallel over batch — core b computes batch element b end to end
(no collectives). kernel() takes full inputs, returns the full output.

Per-core algorithm (Tile framework, one NEFF run SPMD on 8 cores):
  - x is DMA'd with 16 consecutive rows per partition (8 KB contiguous per
    partition, near-peak DMA). This perfectly-shuffles the sequence axis
    (s = 16p + t); attention is permutation-equivariant, so the output DMA
    simply inverts the shuffle.
  - xT = PE-transpose(x tiles); QT/KT = W.T @ xT + b as [d,s] (bf16), V
    directly as [s,d] tiles from xT (bf16, bias via broadcast add).
  - Main loop, software-pipelined over 8 chunks (2 k-tiles) per q-group:
      scoresT[sk,sq] = KT_kt.T @ QT_g   (bf16 matmuls, N=512, psum fp32,
                                         double-buffered 2-bank stage slots)
      PT = exp(scale*scoresT)           (one ScalarE activation per chunk,
                                         psum->sbuf, bf16 out)
      oT += V_kt.T' @ PT                (AV accumulate [d,sq] in psum)
      den: ones.T @ PT                  (4 M=32 col-group-packed matmuls per
                                         2 chunks, concurrent in PE array)
    PE issues scores one chunk ahead of AV/den so it never head-of-line
    blocks on the exp.
  - Epilogue per group: den strips -> sbuf, selector matmuls (sum strips AND
    put q on partitions), one reciprocal [128,16], PE-transpose of oT back to
    [sq,d], per-partition scale by 1/den during the psum->sbuf copy, DMA out.

Numerics: scores/AV in bf16 with fp32 psum accumulation (rel err ~2.6e-3 vs
fp32 reference; exp/softmax denominators in fp32, den reduction in fp32r).
Set USE_BF16_QK=False for fp32r (~fp22) scores (~9e-4 rel err, ~15 us slower).
"""

import numpy as np

S = 2048
D = 128
USE_BF16_QK = True
NT = S // 128          # 16 s-tiles of 128
NG = S // 512          # 4 q-groups of 512
SCALE = float(1.0 / np.sqrt(D))

_PROGRAM = None
LAST_RESULTS = None


def _build():
    from contextlib import ExitStack

    import concourse.bass as bass
    import concourse.mybir as mybir
    import concourse.tile as tile
    from concourse import bacc

    fp32 = mybir.dt.float32
    fp32r = mybir.dt.float32r
    bf16 = mybir.dt.bfloat16
    qkdt = bf16 if USE_BF16_QK else fp32r
    Exp = mybir.ActivationFunctionType.Exp

    nc = bacc.Bacc(trn_type="TRN2", target_bir_lowering=False)

    x_d = nc.dram_tensor("x", [S, D], fp32, kind="ExternalInput").ap()
    w_d = nc.dram_tensor("w3", [D, 3 * D], fp32, kind="ExternalInput").ap()
    # consts layout: [bq | bk | bv | sel(4) | ident(128) | bv_bcast(4x128)] = [128, 647]
    c_d = nc.dram_tensor("consts", [D, 647], fp32, kind="ExternalInput").ap()
    out_d = nc.dram_tensor("out", [S, D], fp32, kind="ExternalOutput").ap()

    # x loaded with 16 consecutive rows per partition (8 KB contiguous per
    # partition -> near-peak DMA). This applies the perfect-shuffle permutation
    # s = 16*p + t to the sequence axis; attention is permutation-equivariant,
    # so we simply invert it when storing the output.
    x_r = x_d.rearrange("(p r) d -> p r d", p=128)
    out_r = out_d.rearrange("(p r) d -> p r d", p=128)

    with tile.TileContext(nc) as tc, ExitStack() as ctx:
        singles = ctx.enter_context(tc.tile_pool(name="singles", bufs=1))
        xin = ctx.enter_context(tc.tile_pool(name="xin", bufs=3))
        ptp = ctx.enter_context(tc.tile_pool(name="pt", bufs=3))
        outp = ctx.enter_context(tc.tile_pool(name="outp", bufs=2))
        # PSUM: stage 4 banks + av 2 + tp/den 2 = 8 banks exactly
        stage_p = ctx.enter_context(tc.tile_pool(name="stage", bufs=1, space="PSUM"))
        av_p = ctx.enter_context(tc.tile_pool(name="av", bufs=1, space="PSUM"))
        tp_p = ctx.enter_context(tc.tile_pool(name="tp", bufs=1, space="PSUM"))

        # --- constants (small consts DMA first: identity gates the transposes) ---
        consts_sb = singles.tile([128, 647], fp32, tag="consts")
        nc.sync.dma_start(out=consts_sb[:, 0:135], in_=c_d[:, 0:135])
        nc.gpsimd.dma_start(out=consts_sb[:, 135:647], in_=c_d[:, 135:647])
        bq_sb = consts_sb[:, 0:1]
        bk_sb = consts_sb[:, 1:2]
        bv_sb = consts_sb[:, 2:3]
        id_sb = consts_sb[:, 7:135]
        bvb_sb = consts_sb[:, 135:647]
        sel_sb = singles.tile([128, 4], fp32r, tag="sel")
        nc.vector.tensor_copy(sel_sb, consts_sb[:, 3:7])
        ones_sb = singles.tile([128, 128], bf16, tag="ones")
        id16_sb = singles.tile([128, 128], bf16, tag="id16")
        ones_stage = xin.tile([128, 128], fp32, tag="wstage")
        nc.vector.memset(ones_stage, 1.0)
        nc.vector.tensor_copy(ones_sb, ones_stage)
        nc.vector.tensor_copy(id16_sb, id_sb)

        # x: 4 quarter-DMAs (2 KB contiguous per partition each) so the
        # transfers spread over multiple HWDGE queues in parallel
        x_q = []
        for h in range(4):
            xh = singles.tile([128, 4, 128], fp32, tag=f"xh{h}", name=f"xh_{h}")
            nc.sync.dma_start(out=xh, in_=x_r[:, 4 * h:4 * (h + 1), :])
            x_q.append(xh)

        w3_stage = singles.tile([128, 384], fp32, tag="w3stage")
        nc.gpsimd.dma_start(out=w3_stage, in_=w_d)
        w3_sb = singles.tile([128, 384], qkdt, tag="w3")
        nc.vector.tensor_copy(w3_sb, w3_stage)
        wq_sb = w3_sb[:, 0:128]
        wk_sb = w3_sb[:, 128:256]
        wv_sb = w3_sb[:, 256:384]

        # --- persistent big sbuf tensors ---
        xT_sb = singles.tile([128, S], qkdt, tag="xT")   # [d, s]
        qT_sb = singles.tile([128, S], qkdt, tag="qT")   # [e, s]
        kT_sb = singles.tile([128, S], qkdt, tag="kT")   # [e, s]
        v_sb = singles.tile([128, S], bf16, tag="v")     # 16 tiles of [s(128), d]

        # per chunk-of-4-tiles: cast x to bf16, transpose, project QT/KT, compute V
        for c in range(4):
            tpt = tp_p.tile([128, 512], fp32, tag=f"tp{c % 2}", name=f"tptx_{c}")
            for j in range(4):
                t = 4 * c + j
                nc.tensor.matmul(
                    tpt[:, 128 * j:128 * (j + 1)], lhsT=x_q[t // 4][:, t % 4, :],
                    rhs=id_sb,
                    is_transpose=True, start=(j == 0), stop=(j == 3),
                )
            nc.vector.tensor_copy(xT_sb[:, 512 * c:512 * (c + 1)], tpt)
        for c in range(4):
            sl = slice(512 * c, 512 * (c + 1))
            for wi, (w_sb, b_sb, dst) in enumerate((
                (wq_sb, bq_sb, qT_sb), (wk_sb, bk_sb, kT_sb),
            )):
                pp = av_p.tile([128, 512], fp32, tag=f"av{wi % 2}", name=f"pp_{c}_{wi}")
                nc.tensor.matmul(pp, lhsT=w_sb, rhs=xT_sb[:, sl],
                                 start=True, stop=True)
                nc.vector.tensor_scalar_add(dst[:, sl], pp, b_sb)
        for c in range(4):
            tpv = tp_p.tile([128, 512], fp32, tag=f"tp{(c + 1) % 2}", name=f"tptv_{c}")
            for j in range(4):
                t = 4 * c + j
                nc.tensor.matmul(
                    tpv[:, 128 * j:128 * (j + 1)],
                    lhsT=xT_sb[:, 128 * t:128 * (t + 1)], rhs=wv_sb,
                    start=(j == 0), stop=(j == 3), skip_group_check=True,
                )
            nc.vector.tensor_add(v_sb[:, 512 * c:512 * (c + 1)], tpv, bvb_sb)

        # --- main attention loop, software-pipelined over 2-k-tile chunks ---
        # Per step: PE issues scores(chunk i) first (stage is double-buffered,
        # so always ready), then AV+den for chunk i-1 (whose exp finished during
        # the previous step). ScalarE exp of chunk i overlaps AV/den of i-1.
        NCH = 8                       # chunks per group, 2 k-tiles each
        chunks = [(g, c) for g in range(NG) for c in range(NCH)]
        av = den = None
        avs, dens, pts = {}, {}, {}

        def issue_scores(g, c):
            st = stage_p.tile([128, 1024], fp32, tag=f"stage{(g * NCH + c) % 2}",
                              name=f"st_{g}_{c}")
            with nc.named_scope("scores"):
                for j in range(2):
                    kt = 2 * c + j
                    nc.tensor.matmul(
                        st[:, 512 * j:512 * (j + 1)],
                        lhsT=kT_sb[:, 128 * kt:128 * (kt + 1)],
                        rhs=qT_sb[:, 512 * g:512 * (g + 1)],
                        start=True, stop=True,
                    )
            pt = ptp.tile([128, 1024], bf16, tag="pt")
            with nc.named_scope("exp"):
                nc.scalar.activation(pt, st, Exp, scale=SCALE)
            return pt

        def issue_den_quad(g, c0):
            # quad covers chunks c0, c0+1 (k-tiles 2*c0 .. 2*c0+3), both pt
            # tiles already materialized -> 4 back-to-back col-group matmuls
            # run concurrently in the PE array.
            with nc.named_scope("den"):
                for q in range(4):
                    kt = 2 * c0 + q
                    ptq, jq = pts[g, c0 + q // 2], kt % 2
                    strip = kt % 4
                    nc.tensor.matmul(
                        dens[g][32 * strip:32 * (strip + 1), :],
                        lhsT=ones_sb[:, 0:32],
                        rhs=ptq[:, 512 * jq:512 * (jq + 1)],
                        start=(c0 == 0), stop=(c0 == NCH - 2),
                        tile_position=(0, 32 * strip),
                        skip_group_check=True,
                    )

        def issue_avden(g, c, pt):
            pts[g, c] = pt
            with nc.named_scope("av"):
                for j in range(2):
                    kt = 2 * c + j
                    nc.tensor.matmul(
                        avs[g], lhsT=v_sb[:, 128 * kt:128 * (kt + 1)],
                        rhs=pt[:, 512 * j:512 * (j + 1)],
                        start=(kt == 0), stop=(kt == 15),
                    )
            if c % 2 == 0 and c > 0:
                issue_den_quad(g, c - 2)

        def epilogue(g):
            av, den = avs.pop(g), dens.pop(g)
            with nc.named_scope("epi"):
                den_fs = outp.tile([128, 512], fp32r, tag="denfs")
                nc.vector.tensor_copy(den_fs, den)
                denT = tp_p.tile([128, 16], fp32, tag=f"tp{(g + 1) % 2}", name=f"denT_{g}")
                for j in range(4):
                    nc.tensor.matmul(
                        denT[:, 4 * j:4 * (j + 1)],
                        lhsT=den_fs[:, 128 * j:128 * (j + 1)],
                        rhs=sel_sb, start=(j == 0), stop=(j == 3),
                    )
                recip = outp.tile([128, 16], fp32, tag="recip")
                nc.vector.reciprocal(recip, denT)
                oT_sb = outp.tile([128, 512], bf16, tag="oTsb")
                nc.vector.tensor_copy(oT_sb, av)
                tpo = tp_p.tile([128, 512], bf16, tag=f"tp{g % 2}", name=f"tpo_{g}")
                for j in range(4):
                    nc.tensor.matmul(
                        tpo[:, 128 * j:128 * (j + 1)],
                        lhsT=oT_sb[:, 128 * j:128 * (j + 1)], rhs=id16_sb,
                        is_transpose=True, start=(j == 0), stop=(j == 3),
                    )
                osb = outp.tile([128, 512], fp32, tag="osb")
                for j in range(4):
                    nc.vector.tensor_scalar_mul(
                        osb[:, 128 * j:128 * (j + 1)],
                        tpo[:, 128 * j:128 * (j + 1)], recip[:, 4 * j:4 * j + 1],
                    )
                nc.sync.dma_start(
                    out=out_r[:, 4 * g:4 * (g + 1), :],
                    in_=osb.rearrange("p (j d) -> p j d", j=4),
                )

        prev = None
        for g, c in chunks:
            if c == 0:
                avs[g] = av_p.tile([128, 512], fp32, tag=f"av{g % 2}", name=f"av_{g}")
                dens[g] = tp_p.tile([128, 512], fp32, tag=f"tp{g % 2}", name=f"den_{g}")
            pt = issue_scores(g, c)
            if prev is not None:
                issue_avden(*prev)
                if prev[1] == NCH - 1:
                    issue_den_quad(prev[0], NCH - 2)
                    epilogue(prev[0])
            prev = (g, c, pt)
        issue_avden(*prev)
        issue_den_quad(prev[0], NCH - 2)
        epilogue(prev[0])

    nc.compile()
    return nc


def _get_program():
    global _PROGRAM
    if _PROGRAM is None:
        _PROGRAM = _build()
    return _PROGRAM


def _ensure_axon_hooks():
    """bass_utils imports antenv.axon_hooks when tracing; provide a stub if
    the image's antenv lacks it (hook defaults to None => tracing skipped)."""
    import sys
    import types
    try:
        import antenv.axon_hooks  # noqa: F401
        return
    except ImportError:
        pass
    import antenv
    m = types.ModuleType("antenv.axon_hooks")
    m._hook = None
    def _set(h):
        m._hook = h
    def _get():
        return m._hook
    m.set_axon_ntff_profile_hook = _set
    m.get_axon_ntff_profile_hook = _get
    sys.modules["antenv.axon_hooks"] = m
    antenv.axon_hooks = m


def kernel(input1, Wq, bq, Wk, bk, Wv, bv):
    global LAST_RESULTS
    _ensure_axon_hooks()
    from concourse.bass_utils import run_bass_kernel_spmd

    nc = _get_program()

    input1 = np.ascontiguousarray(np.asarray(input1, dtype=np.float32))
    w3 = np.concatenate([np.asarray(W, np.float32).T for W in (Wq, Wk, Wv)],
                        axis=1)
    sel = np.tile(np.array([1.0 if p % 32 == 0 else 0.0 for p in range(D)],
                  np.float32).reshape(D, 1), (1, 4))
    consts = np.concatenate([
        np.asarray(bq, np.float32).reshape(D, 1),
        np.asarray(bk, np.float32).reshape(D, 1),
        np.asarray(bv, np.float32).reshape(D, 1),
        sel,
        np.eye(D, dtype=np.float32),
        np.tile(np.asarray(bv, np.float32).reshape(1, D), (D, 4)),
    ], axis=1)
    common = {
        "w3": np.ascontiguousarray(w3),
        "consts": np.ascontiguousarray(consts),
    }
    in_maps = [dict(common, x=input1[b]) for b in range(8)]
    res = run_bass_kernel_spmd(nc, in_maps, core_ids=list(range(8)))
    LAST_RESULTS = res
    return np.stack([r["out"] for r in res.results], axis=0)


# revision 39
# speedup vs baseline: 1.8557x; 1.0111x over previous
"""Single-head attention (B=8, S=2048, D=128) on 8 Trainium2 NeuronCores.

Sharding: data-parallel over batch — core b computes batch element b end to end
(no collectives). kernel() takes full inputs, returns the full output.

Per-core algorithm (Tile framework, one NEFF run SPMD on 8 cores):
  - x is DMA'd with 16 consecutive rows per partition (8 KB contiguous per
    partition, near-peak DMA). This perfectly-shuffles the sequence axis
    (s = 16p + t); attention is permutation-equivariant, so the output DMA
    simply inverts the shuffle.
  - xT = PE-transpose(x tiles); QT/KT = W.T @ xT + b as [d,s] (bf16), V
    directly as [s,d] tiles from xT (bf16, bias via broadcast add).
  - Main loop, software-pipelined over 8 chunks (2 k-tiles) per q-group:
      scoresT[sk,sq] = KT_kt.T @ QT_g   (bf16 matmuls, N=512, psum fp32,
                                         double-buffered 2-bank stage slots)
      PT = exp(scale*scoresT)           (one ScalarE activation per chunk,
                                         psum->sbuf, bf16 out)
      oT += V_kt.T' @ PT                (AV accumulate [d,sq] in psum)
      den: ones.T @ PT                  (4 M=32 col-group-packed matmuls per
                                         2 chunks, concurrent in PE array)
    PE issues scores one chunk ahead of AV/den so it never head-of-line
    blocks on the exp.
  - Epilogue per group: den strips -> sbuf, selector matmuls (sum strips AND
    put q on partitions), one reciprocal [128,16], PE-transpose of oT back to
    [sq,d], per-partition scale by 1/den during the psum->sbuf copy, DMA out.

Numerics: scores/AV in bf16 with fp32 psum accumulation (rel err ~2.6e-3 vs
fp32 reference; exp/softmax denominators in fp32, den reduction in fp32r).
Set USE_BF16_QK=False for fp32r (~fp22) scores (~9e-4 rel err, ~15 us slower).
"""

import numpy as np

S = 2048
D = 128
USE_BF16_QK = True
NT = S // 128          # 16 s-tiles of 128
NG = S // 512          # 4 q-groups of 512
SCALE = float(1.0 / np.sqrt(D))

_PROGRAM = None
LAST_RESULTS = None


def _build():
    from contextlib import ExitStack

    import concourse.bass as bass
    import concourse.mybir as mybir
    import concourse.tile as tile
    from concourse import bacc

    fp32 = mybir.dt.float32
    fp32r = mybir.dt.float32r
    bf16 = mybir.dt.bfloat16
    qkdt = bf16 if USE_BF16_QK else fp32r
    Exp = mybir.ActivationFunctionType.Exp

    nc = bacc.Bacc(trn_type="TRN2", target_bir_lowering=False)

    x_d = nc.dram_tensor("x", [S, D], fp32, kind="ExternalInput").ap()
    w_d = nc.dram_tensor("w3", [D, 3 * D], fp32, kind="ExternalInput").ap()
    # consts layout: [bq | bk | bv | sel(4) | ident(128) | bv_bcast(4x128)] = [128, 647]
    c_d = nc.dram_tensor("consts", [D, 647], fp32, kind="ExternalInput").ap()
    out_d = nc.dram_tensor("out", [S, D], fp32, kind="ExternalOutput").ap()

    # x loaded with 16 consecutive rows per partition (8 KB contiguous per
    # partition -> near-peak DMA). This applies the perfect-shuffle permutation
    # s = 16*p + t to the sequence axis; attention is permutation-equivariant,
    # so we simply invert it when storing the output.
    x_r = x_d.rearrange("(p r) d -> p r d", p=128)
    out_r = out_d.rearrange("(p r) d -> p r d", p=128)

    with tile.TileContext(nc) as tc, ExitStack() as ctx:
        singles = ctx.enter_context(tc.tile_pool(name="singles", bufs=1))
        xin = ctx.enter_context(tc.tile_pool(name="xin", bufs=3))
        ptp = ctx.enter_context(tc.tile_pool(name="pt", bufs=4))
        outp = ctx.enter_context(tc.tile_pool(name="outp", bufs=2))
        # PSUM: stage 4 banks + av 2 + tp/den 2 = 8 banks exactly
        stage_p = ctx.enter_context(tc.tile_pool(name="stage", bufs=1, space="PSUM"))
        av_p = ctx.enter_context(tc.tile_pool(name="av", bufs=1, space="PSUM"))
        tp_p = ctx.enter_context(tc.tile_pool(name="tp", bufs=1, space="PSUM"))

        # --- constants (small consts DMA first: identity gates the transposes) ---
        consts_sb = singles.tile([128, 647], fp32, tag="consts")
        nc.sync.dma_start(out=consts_sb[:, 0:135], in_=c_d[:, 0:135])
        nc.gpsimd.dma_start(out=consts_sb[:, 135:647], in_=c_d[:, 135:647])
        bq_sb = consts_sb[:, 0:1]
        bk_sb = consts_sb[:, 1:2]
        bv_sb = consts_sb[:, 2:3]
        id_sb = consts_sb[:, 7:135]
        bvb_sb = consts_sb[:, 135:647]
        sel_sb = singles.tile([128, 4], fp32r, tag="sel")
        nc.vector.tensor_copy(sel_sb, consts_sb[:, 3:7])
        ones_sb = singles.tile([128, 128], bf16, tag="ones")
        id16_sb = singles.tile([128, 128], bf16, tag="id16")
        ones_stage = xin.tile([128, 128], fp32, tag="wstage")
        nc.vector.memset(ones_stage, 1.0)
        nc.vector.tensor_copy(ones_sb, ones_stage)
        nc.vector.tensor_copy(id16_sb, id_sb)

        # x: 4 quarter-DMAs (2 KB contiguous per partition each) so the
        # transfers spread over multiple HWDGE queues in parallel
        x_q = []
        for h in range(4):
            xh = singles.tile([128, 4, 128], fp32, tag=f"xh{h}", name=f"xh_{h}")
            nc.sync.dma_start(out=xh, in_=x_r[:, 4 * h:4 * (h + 1), :])
            x_q.append(xh)

        w3_stage = singles.tile([128, 384], fp32, tag="w3stage")
        nc.gpsimd.dma_start(out=w3_stage, in_=w_d)
        w3_sb = singles.tile([128, 384], qkdt, tag="w3")
        nc.vector.tensor_copy(w3_sb, w3_stage)
        wq_sb = w3_sb[:, 0:128]
        wk_sb = w3_sb[:, 128:256]
        wv_sb = w3_sb[:, 256:384]

        # --- persistent big sbuf tensors ---
        xT_sb = singles.tile([128, S], qkdt, tag="xT")   # [d, s]
        qT_sb = singles.tile([128, S], qkdt, tag="qT")   # [e, s]
        kT_sb = singles.tile([128, S], qkdt, tag="kT")   # [e, s]
        v_sb = singles.tile([128, S], bf16, tag="v")     # 16 tiles of [s(128), d]

        # per chunk-of-4-tiles: cast x to bf16, transpose, project QT/KT, compute V
        for c in range(4):
            tpt = tp_p.tile([128, 512], fp32, tag=f"tp{c % 2}", name=f"tptx_{c}")
            for j in range(4):
                t = 4 * c + j
                nc.tensor.matmul(
                    tpt[:, 128 * j:128 * (j + 1)], lhsT=x_q[t // 4][:, t % 4, :],
                    rhs=id_sb,
                    is_transpose=True, start=(j == 0), stop=(j == 3),
                )
            nc.vector.tensor_copy(xT_sb[:, 512 * c:512 * (c + 1)], tpt)
        for c in range(4):
            sl = slice(512 * c, 512 * (c + 1))
            for wi, (w_sb, b_sb, dst) in enumerate((
                (wq_sb, bq_sb, qT_sb), (wk_sb, bk_sb, kT_sb),
            )):
                pp = av_p.tile([128, 512], fp32, tag=f"av{wi % 2}", name=f"pp_{c}_{wi}")
                nc.tensor.matmul(pp, lhsT=w_sb, rhs=xT_sb[:, sl],
                                 start=True, stop=True)
                nc.vector.tensor_scalar_add(dst[:, sl], pp, b_sb)
        for c in range(4):
            tpv = tp_p.tile([128, 512], fp32, tag=f"tp{(c + 1) % 2}", name=f"tptv_{c}")
            for j in range(4):
                t = 4 * c + j
                nc.tensor.matmul(
                    tpv[:, 128 * j:128 * (j + 1)],
                    lhsT=xT_sb[:, 128 * t:128 * (t + 1)], rhs=wv_sb,
                    start=(j == 0), stop=(j == 3), skip_group_check=True,
                )
            nc.vector.tensor_add(v_sb[:, 512 * c:512 * (c + 1)], tpv, bvb_sb)

        # --- main attention loop, software-pipelined over 2-k-tile chunks ---
        # Per step: PE issues scores(chunk i) first (stage is double-buffered,
        # so always ready), then AV+den for chunk i-1 (whose exp finished during
        # the previous step). ScalarE exp of chunk i overlaps AV/den of i-1.
        NCH = 8                       # chunks per group, 2 k-tiles each
        chunks = [(g, c) for g in range(NG) for c in range(NCH)]
        av = den = None
        avs, dens, pts = {}, {}, {}

        def issue_scores(g, c):
            st = stage_p.tile([128, 1024], fp32, tag=f"stage{(g * NCH + c) % 2}",
                              name=f"st_{g}_{c}")
            with nc.named_scope("scores"):
                for j in range(2):
                    kt = 2 * c + j
                    nc.tensor.matmul(
                        st[:, 512 * j:512 * (j + 1)],
                        lhsT=kT_sb[:, 128 * kt:128 * (kt + 1)],
                        rhs=qT_sb[:, 512 * g:512 * (g + 1)],
                        start=True, stop=True,
                    )
            pt = ptp.tile([128, 1024], bf16, tag=f"pt{(g * NCH + c) % 2}", name=f"pt_{g}_{c}", bufs=2)
            with nc.named_scope("exp"):
                nc.scalar.activation(pt, st, Exp, scale=SCALE)
            return pt

        def issue_den_quad(g, c0):
            # quad covers chunks c0, c0+1 (k-tiles 2*c0 .. 2*c0+3), both pt
            # tiles already materialized -> 4 back-to-back col-group matmuls
            # run concurrently in the PE array.
            with nc.named_scope("den"):
                for q in range(4):
                    kt = 2 * c0 + q
                    ptq, jq = pts[g, c0 + q // 2], kt % 2
                    strip = kt % 4
                    nc.tensor.matmul(
                        dens[g][32 * strip:32 * (strip + 1), :],
                        lhsT=ones_sb[:, 0:32],
                        rhs=ptq[:, 512 * jq:512 * (jq + 1)],
                        start=(c0 == 0), stop=(c0 == NCH - 2),
                        tile_position=(0, 32 * strip),
                        skip_group_check=True,
                    )

        def issue_avden(g, c, pt):
            pts[g, c] = pt
            with nc.named_scope("av"):
                for j in range(2):
                    kt = 2 * c + j
                    nc.tensor.matmul(
                        avs[g], lhsT=v_sb[:, 128 * kt:128 * (kt + 1)],
                        rhs=pt[:, 512 * j:512 * (j + 1)],
                        start=(kt == 0), stop=(kt == 15),
                    )
            if c % 2 == 0 and c > 0:
                issue_den_quad(g, c - 2)

        def epilogue(g):
            av, den = avs.pop(g), dens.pop(g)
            with nc.named_scope("epi"):
                den_fs = outp.tile([128, 512], fp32r, tag=f"denfs{g % 2}", name=f"denfs_{g}", bufs=1)
                nc.vector.tensor_copy(den_fs, den)
                denT = tp_p.tile([128, 16], fp32, tag=f"tp{(g + 1) % 2}", name=f"denT_{g}")
                for j in range(4):
                    nc.tensor.matmul(
                        denT[:, 4 * j:4 * (j + 1)],
                        lhsT=den_fs[:, 128 * j:128 * (j + 1)],
                        rhs=sel_sb, start=(j == 0), stop=(j == 3),
                    )
                recip = outp.tile([128, 16], fp32, tag=f"recip{g % 2}", name=f"recip_{g}", bufs=1)
                nc.vector.reciprocal(recip, denT)
                oT_sb = outp.tile([128, 512], bf16, tag=f"oTsb{g % 2}", name=f"oTsb_{g}", bufs=1)
                nc.vector.tensor_copy(oT_sb, av)
                tpo = tp_p.tile([128, 512], bf16, tag=f"tp{g % 2}", name=f"tpo_{g}")
                for j in range(4):
                    nc.tensor.matmul(
                        tpo[:, 128 * j:128 * (j + 1)],
                        lhsT=oT_sb[:, 128 * j:128 * (j + 1)], rhs=id16_sb,
                        is_transpose=True, start=(j == 0), stop=(j == 3),
                    )
                osb = outp.tile([128, 512], fp32, tag=f"osb{g % 2}", name=f"osb_{g}", bufs=1)
                for j in range(4):
                    nc.vector.tensor_scalar_mul(
                        osb[:, 128 * j:128 * (j + 1)],
                        tpo[:, 128 * j:128 * (j + 1)], recip[:, 4 * j:4 * j + 1],
                    )
                nc.sync.dma_start(
                    out=out_r[:, 4 * g:4 * (g + 1), :],
                    in_=osb.rearrange("p (j d) -> p j d", j=4),
                )

        prev = None
        for g, c in chunks:
            if c == 0:
                avs[g] = av_p.tile([128, 512], fp32, tag=f"av{g % 2}", name=f"av_{g}")
                dens[g] = tp_p.tile([128, 512], fp32, tag=f"tp{g % 2}", name=f"den_{g}")
            pt = issue_scores(g, c)
            if prev is not None:
                issue_avden(*prev)
                if prev[1] == NCH - 1:
                    issue_den_quad(prev[0], NCH - 2)
                    epilogue(prev[0])
            prev = (g, c, pt)
        issue_avden(*prev)
        issue_den_quad(prev[0], NCH - 2)
        epilogue(prev[0])

    nc.compile()
    return nc


def _get_program():
    global _PROGRAM
    if _PROGRAM is None:
        _PROGRAM = _build()
    return _PROGRAM


def _ensure_axon_hooks():
    """bass_utils imports antenv.axon_hooks when tracing; provide a stub if
    the image's antenv lacks it (hook defaults to None => tracing skipped)."""
    import sys
    import types
    try:
        import antenv.axon_hooks  # noqa: F401
        return
    except ImportError:
        pass
    import antenv
    m = types.ModuleType("antenv.axon_hooks")
    m._hook = None
    def _set(h):
        m._hook = h
    def _get():
        return m._hook
    m.set_axon_ntff_profile_hook = _set
    m.get_axon_ntff_profile_hook = _get
    sys.modules["antenv.axon_hooks"] = m
    antenv.axon_hooks = m


def kernel(input1, Wq, bq, Wk, bk, Wv, bv):
    global LAST_RESULTS
    _ensure_axon_hooks()
    from concourse.bass_utils import run_bass_kernel_spmd

    nc = _get_program()

    input1 = np.ascontiguousarray(np.asarray(input1, dtype=np.float32))
    w3 = np.concatenate([np.asarray(W, np.float32).T for W in (Wq, Wk, Wv)],
                        axis=1)
    sel = np.tile(np.array([1.0 if p % 32 == 0 else 0.0 for p in range(D)],
                  np.float32).reshape(D, 1), (1, 4))
    consts = np.concatenate([
        np.asarray(bq, np.float32).reshape(D, 1),
        np.asarray(bk, np.float32).reshape(D, 1),
        np.asarray(bv, np.float32).reshape(D, 1),
        sel,
        np.eye(D, dtype=np.float32),
        np.tile(np.asarray(bv, np.float32).reshape(1, D), (D, 4)),
    ], axis=1)
    common = {
        "w3": np.ascontiguousarray(w3),
        "consts": np.ascontiguousarray(consts),
    }
    in_maps = [dict(common, x=input1[b]) for b in range(8)]
    res = run_bass_kernel_spmd(nc, in_maps, core_ids=list(range(8)))
    LAST_RESULTS = res
    return np.stack([r["out"] for r in res.results], axis=0)
